# revision 18
# baseline (speedup 1.0000x reference)
"""ChannelBlockImportanceGate kernel for 8 Trainium2 NeuronCores.

Computes, per (b, c) slice of features [8, 256, 132, 132] f32:
  scores = block-sum of |x| over 8x8 blocks (17x17 grid, zero-padded edges)
  top-72 blocks (ties -> lowest index, matching jax.lax.top_k)
  output = per-pixel {0,1} mask upsampled 8x8 (cropped to 132x132)

The straight-through soft term of the reference cancels in the forward
pass (hard - sg(soft) + soft == hard up to ~1ulp), so the output is the
hard mask.

Sharding: purely data parallel. 2048 (b,c) slices -> 256 per core.
Per core: 2 groups of 128 slices; each slice occupies one SBUF
partition so pooling/topk/upsample are per-partition ops with no
cross-partition traffic. Top-72 uses 9 rounds of DVE max8 +
match_replace(-1e30), then mask = (score < 0).

Performance model (from extensive perfetto analysis + microbenches):
 - The kernel is DMA-bound: 17.8MB load + 17.8MB store per core at
   the ~26.4GB/s per-SDMA-engine SBUF-port rate = ~84.5us of busy
   time on each of the 16 engines. A pure DMA round-trip of the same
   bytes measures 96-118us depending on an ENVIRONMENTAL mode (SDMA
   engine 15 sporadically runs ~20% slower, minutes-timescale; not
   controllable from the kernel). The full kernel measures within
   +-noise of that pure-DMA control, i.e. all compute is hidden.
 - Cast-during-DMA does NOT reduce engine busy time (engine time is
   set by the f32 side -- measured), but the PACKED fp8 mask makes
   the upsample 4x cheaper on the compute engines: one f32 element =
   4 fp8(0|1) pixels (mask * 0x38383838-as-f32, exact since mask is
   {0,1}); SWDGE stores cast fp8->f32 bit-exactly into the output.
 - DMAs are spread across all three queues (loads alternate the two
   HWDGE queues, stores ride the Pool/SWDGE queue): per-DMA
   completion overhead serializes within a queue but overlaps across
   queues (measured 117us -> 97us for the same 24 chunked DMAs).
 - Vector chain: pool g0 (18.7us, arrival-paced chunks) -> topk g0
   (9.5us) -> pool g1 -> topk g1, finishing ~77us so the g1 store
   stream (the last ~21us of per-engine DMA work) starts ~79us and
   the kernel lands ~104us in the clean mode, ~110-118 in the slow
   environmental mode.
 - GpSimd does NO compute (only SWDGE store emission); scalar does
   all upsample copies; vector does pooling/topk/masks.
"""

import numpy as np

B, C, H, W = 8, 256, 132, 132
HW = H * W            # 17424
NB = 17               # 8x8 blocks per side (132 padded to 136)
NBLK = NB * NB        # 289
KEEP = 72             # round(289 * 0.25)
N_CORES = 8
S = (B * C) // N_CORES  # 256 slices per core
WP = W // 4           # 33 packed u32 per pixel row
LOAD_CHUNKS = ((0, 16), (16, 48), (48, 80), (80, 104), (104, 120),
               (120, 132))
# Store chunks per group: SWDGE descriptor emission is ~1us per DMA,
# serial on the Q7 core, so many small stores trickle descriptors and
# leave the SDMA engines idle at stream-start. g0's stores start while
# loads still run (granularity irrelevant) -> 2 big chunks; g1's first
# chunk is small so its store stream starts right after topk g1.
STORE_CHUNKS_BY_GROUP = (((0, 64), (64, 132)),
                         ((0, 16), (16, 64), (64, 132)))
NEG = -1.0e30
# f32 whose 4 bytes are each fp8e4m3(1.0) = 0x38; mask * PACK4 produces
# the packed 4-pixel fp8 row exactly (mask is exactly 0.0 or 1.0).
PACK4 = float(np.frombuffer(np.uint32(0x38383838).tobytes(),
                            dtype=np.float32)[0])

_prog_cache = {}


def _build_program():
    import concourse.bacc as bacc
    import concourse.mybir as mybir
    import concourse.tile as tile

    f32 = mybir.dt.float32
    fp8 = mybir.dt.float8e4
    X = mybir.AxisListType.X
    XY = mybir.AxisListType.XY
    ADD = mybir.AluOpType.add

    nc = bacc.Bacc("TRN2", debug=False, num_devices=N_CORES)
    x = nc.dram_tensor("x", (S, HW), f32, kind="ExternalInput")
    y = nc.dram_tensor("y", (S, HW), f32, kind="ExternalOutput")

    with tile.TileContext(nc) as tc:
        with (
            tc.tile_pool(name="big", bufs=2) as bigp,
            tc.tile_pool(name="med", bufs=2) as medp,
            tc.tile_pool(name="small", bufs=2) as smallp,
        ):
            # All load DMAs are emitted before any store DMA. Loads ride
            # the sync HWDGE queue (g0's first chunk on the scalar HWDGE
            # queue, measured fastest in v1); stores ride the Pool
            # (SWDGE) queue so load/store packets interleave round-robin
            # on each SDMA engine.
            xb = []
            li = 0
            for g in range(S // 128):
                p0 = g * 128
                xt = bigp.tile([128, HW], f32, name=f"xb_g{g}", tag="xb")
                for k, (r0, r1) in enumerate(LOAD_CHUNKS):
                    # Alternate the two HWDGE queues: per-DMA completion
                    # overhead serializes within a queue but overlaps
                    # across queues (mb2 vs mb4: 117us -> 97us for the
                    # same 24 chunked DMAs).
                    eng = nc.sync if li % 2 == 0 else nc.scalar
                    li += 1
                    eng.dma_start(out=xt[:, r0 * W:r1 * W],
                                  in_=x[p0:p0 + 128, r0 * W:r1 * W])
                xb.append(xt)

            xv = [xt.rearrange("p (r w) -> p r w", w=W) for xt in xb]
            scores = [smallp.tile([128, NBLK], f32, name=f"scores_g{g}",
                                  tag="scores") for g in range(2)]
            sc3 = [s.rearrange("p (h t) -> p h t", t=NB) for s in scores]

            def pool_chunk(g, r0, r1):
                rr1 = min(r1, 128)
                nc.vector.tensor_reduce(
                    out=sc3[g][:, r0 // 8:rr1 // 8, 0:16],
                    in_=(xv[g][:, r0:rr1, 0:128]
                         .rearrange("p (h r) (q c) -> p h q r c",
                                    r=8, c=8)),
                    axis=XY, op=ADD, apply_absolute_value=True)

            def pool_edges(g):
                nc.vector.tensor_reduce(
                    out=sc3[g][:, 0:16, 16:17],
                    in_=(xv[g][:, 0:128, 128:132]
                         .rearrange("p (h r) c -> p h r c", r=8)),
                    axis=XY, op=ADD, apply_absolute_value=True)
                nc.vector.tensor_reduce(
                    out=sc3[g][:, 16:17, 0:16],
                    in_=(xv[g][:, 128:132, 0:128]
                         .rearrange("p r (q c) -> p q r c", c=8)),
                    axis=XY, op=ADD, apply_absolute_value=True)
                nc.vector.tensor_reduce(
                    out=sc3[g][:, 16:17, 16:17],
                    in_=xv[g][:, 128:132, 128:132].unsqueeze(1),
                    axis=XY, op=ADD, apply_absolute_value=True)

            def topk(g, first_m8):
                # Top-72 per partition: 9 rounds of max8 + match_replace.
                # match_replace replaces the first unmatched occurrence,
                # so ties resolve to the lowest index like jax.lax.top_k.
                for it in range(KEEP // 8):
                    m8 = first_m8 if it == 0 else smallp.tile(
                        [128, 8], f32, name=f"m8_g{g}i{it}", tag="m8")
                    nc.vector.max(out=m8[:, :], in_=scores[g][:, :])
                    nc.vector.match_replace(out=scores[g][:, :],
                                            in_to_replace=m8[:, :],
                                            in_values=scores[g][:, :],
                                            imm_value=NEG)

            def pm_rm(g):
                # Packed block mask: replaced entries are -1e30, so
                # (score < 0) * PACK4 writes 0x38383838 (4 fp8 ones) for
                # selected blocks, 0.0 for the rest. Then the packed
                # row-mask [p, h, 33]: one 132-px row (33 packed elems)
                # per block-row; blocks 0..15 span 2 packed elems each,
                # edge block 16 exactly 1 (pixels 128-131).
                pm = smallp.tile([128, NBLK], f32, name=f"pm_g{g}",
                                 tag="pm")
                nc.vector.tensor_scalar(out=pm[:, :], in0=scores[g][:, :],
                                        scalar1=0.0, scalar2=PACK4,
                                        op0=mybir.AluOpType.is_lt,
                                        op1=mybir.AluOpType.mult)
                pm3 = pm.rearrange("p (h t) -> p h t", t=NB)
                rm = medp.tile([128, NB * WP], f32, name=f"rm_g{g}",
                               tag="rm")
                rm3 = rm.rearrange("p (h w) -> p h w", w=WP)
                nc.vector.tensor_copy(
                    out=rm3[:, :, 0:32].rearrange("p h (q c) -> p h q c",
                                                  c=2),
                    in_=(pm3[:, :, 0:16].unsqueeze(3)
                         .broadcast_to((128, NB, 16, 2))))
                nc.vector.tensor_copy(
                    out=rm3[:, :, 32:33],
                    in_=pm3[:, :, 16:17])
                return pm, rm3

            def upsample_store(g, rm3):
                # Vertical 8x upsample into the packed mask tile (scalar),
                # then SWDGE cast-store (fp8 -> f32) per chunk.
                p0 = g * 128
                mk = medp.tile([128, NB * 8 * WP], f32, name=f"mk_g{g}",
                               tag="mk")
                mk4 = mk.rearrange("p (h r w) -> p h r w", r=8, w=WP)
                chunks = STORE_CHUNKS_BY_GROUP[g]
                for k, (r0, r1) in enumerate(chunks):
                    h0, h1 = r0 // 8, (r1 + 7) // 8
                    src = (rm3[:, h0:h1, :].unsqueeze(2)
                           .broadcast_to((128, h1 - h0, 8, WP)))
                    nc.scalar.copy(out=mk4[:, h0:h1, :, :], in_=src)
                    # Store rows r0:r1 (crop block-row 16 to 4 rows via
                    # the flat view of mk).
                    nc.gpsimd.dma_start(
                        out=y[p0:p0 + 128, r0 * W:r1 * W],
                        in_=mk[:, r0 * WP:r1 * WP].bitcast(fp8))

            # Vector-chain order (the tail is store-bound; g0's stores
            # are held to t_L anyway, so topk g0 is NOT urgent -- what
            # matters is topk g1 finishing ASAP after the last load):
            #   pool g0 (arrival-paced) -> pool g1 chunk0 -> topk g0 ->
            #   pm/rm g0 -> pool g1 rest -> topk g1 -> pm/rm g1
            for (r0, r1) in LOAD_CHUNKS:
                pool_chunk(0, r0, r1)
            pool_edges(0)
            m8_g0 = smallp.tile([128, 8], f32, name="m8_g0i0", tag="m8")
            topk(0, m8_g0)
            pm0, rm3_0 = pm_rm(0)
            # Ordering token: pins g1's pooling after g0's mask on the
            # vector engine, otherwise the scheduler interleaves the two
            # groups' pooling and delays g0's mask by ~20us.
            nc.vector.tensor_copy(
                out=scores[1][0:1, :],
                in_=pm0[0:1, 0:1].broadcast_to((1, NBLK)))
            for (r0, r1) in LOAD_CHUNKS:
                pool_chunk(1, r0, r1)
            pool_edges(1)
            m8_g1 = smallp.tile([128, 8], f32, name="m8_g1i0", tag="m8")
            topk(1, m8_g1)
            pm1, rm3_1 = pm_rm(1)

            upsample_store(0, rm3_0)
            upsample_store(1, rm3_1)
    nc.compile()
    return nc


def _ensure_ntff_hook_module():
    """bass_utils' trace path does `from antenv.axon_hooks import
    get_axon_ntff_profile_hook` — a module this image doesn't ship.
    Register an equivalent (ctypes into libaxon_pjrt.so, mirroring
    trn_boot._ntff_profile_via_ctypes) so BASS_TRACE=1 works; degrade
    to a None hook (trace skipped) when unavailable."""
    import sys
    import types

    try:
        import antenv.axon_hooks  # noqa: F401
        return
    except Exception:
        pass

    hook = None
    try:
        import contextlib
        import ctypes

        so_path = "/opt/axon/libaxon_pjrt.so"
        lib = ctypes.CDLL(so_path)
        if hasattr(lib, "axon_start_nrt_profile"):
            lib.axon_start_nrt_profile.argtypes = [
                ctypes.POINTER(ctypes.c_int64), ctypes.c_size_t]
            lib.axon_start_nrt_profile.restype = ctypes.c_int64
            lib.axon_stop_nrt_profile.argtypes = [ctypes.c_char_p]
            lib.axon_stop_nrt_profile.restype = ctypes.c_int64

            @contextlib.contextmanager
            def _hook(output_dir, device_ids):
                import jax
                jax.devices()
                if device_ids:
                    ids = (ctypes.c_int64 * len(device_ids))(*device_ids)
                    rc = lib.axon_start_nrt_profile(ids, len(device_ids))
                else:
                    rc = lib.axon_start_nrt_profile(None, 0)
                if rc != 0:
                    raise RuntimeError(f"axon_start_nrt_profile rc={rc}")
                try:
                    yield
                finally:
                    n = lib.axon_stop_nrt_profile(str(output_dir).encode())
                    print(f"ntff profile: {n} file(s) -> {output_dir}",
                          file=sys.stderr)

            hook = _hook
    except Exception:
        hook = None

    mod = types.ModuleType("antenv.axon_hooks")
    mod.get_axon_ntff_profile_hook = lambda: hook
    mod.set_axon_ntff_profile_hook = lambda h: None
    sys.modules["antenv.axon_hooks"] = mod


def _get_program():
    if "nc" not in _prog_cache:
        _prog_cache["nc"] = _build_program()
    return _prog_cache["nc"]


def kernel(features, enabled):
    feats = np.asarray(features)
    if not bool(np.asarray(enabled)):
        return np.ones(feats.shape, dtype=np.float32)

    _ensure_ntff_hook_module()
    import concourse.bass_utils as _bu
    from concourse.bass_utils import run_bass_kernel_spmd

    # The trace path uploads artifacts to a shared bucket; tolerate
    # sandboxes where that fails.
    if not getattr(_bu, "_upload_patched", False):
        _orig_upload = _bu.upload_artifacts

        def _safe_upload(tmpdir):
            try:
                return _orig_upload(tmpdir)
            except Exception:
                return str(tmpdir)

        _bu.upload_artifacts = _safe_upload
        _bu._upload_patched = True

    nc = _get_program()
    flat = np.ascontiguousarray(feats.reshape(B * C, HW), dtype=np.float32)
    in_maps = [{"x": flat[i * S:(i + 1) * S]} for i in range(N_CORES)]
    res = run_bass_kernel_spmd(nc, in_maps, list(range(N_CORES)))
    _prog_cache["last_res"] = res
    out = np.concatenate([np.asarray(res.results[i]["y"])
                          for i in range(N_CORES)], axis=0)
    return out.reshape(B, C, H, W).astype(np.float32)


# revision 20
# speedup vs baseline: 1.0292x; 1.0292x over previous
"""ChannelBlockImportanceGate kernel for 8 Trainium2 NeuronCores.

Computes, per (b, c) slice of features [8, 256, 132, 132] f32:
  scores = block-sum of |x| over 8x8 blocks (17x17 grid, zero-padded edges)
  top-72 blocks (ties -> lowest index, matching jax.lax.top_k)
  output = per-pixel {0,1} mask upsampled 8x8 (cropped to 132x132)

The straight-through soft term of the reference cancels in the forward
pass (hard - sg(soft) + soft == hard up to ~1ulp), so the output is the
hard mask.

Sharding: purely data parallel. 2048 (b,c) slices -> 256 per core.
Per core: 2 groups of 128 slices; each slice occupies one SBUF
partition so pooling/topk/upsample are per-partition ops with no
cross-partition traffic. Top-72 uses 9 rounds of DVE max8 +
match_replace(-1e30), then mask = (score < 0).

Performance model (from extensive perfetto analysis + microbenches):
 - The kernel is DMA-bound: 17.8MB load + 17.8MB store per core at
   the ~26.4GB/s per-SDMA-engine SBUF-port rate = ~84.5us of busy
   time on each of the 16 engines. A pure DMA round-trip of the same
   bytes measures 96-118us depending on an ENVIRONMENTAL mode (SDMA
   engine 15 sporadically runs ~20% slower, minutes-timescale; not
   controllable from the kernel). The full kernel measures within
   +-noise of that pure-DMA control, i.e. all compute is hidden.
 - Cast-during-DMA does NOT reduce engine busy time (engine time is
   set by the f32 side -- measured), but the PACKED fp8 mask makes
   the upsample 4x cheaper on the compute engines: one f32 element =
   4 fp8(0|1) pixels (mask * 0x38383838-as-f32, exact since mask is
   {0,1}); SWDGE stores cast fp8->f32 bit-exactly into the output.
 - DMAs are spread across all three queues (loads alternate the two
   HWDGE queues, stores ride the Pool/SWDGE queue): per-DMA
   completion overhead serializes within a queue but overlaps across
   queues (measured 117us -> 97us for the same 24 chunked DMAs).
 - Vector chain: pool g0 (18.7us, arrival-paced chunks) -> topk g0
   (9.5us) -> pool g1 -> topk g1, finishing ~77us so the g1 store
   stream (the last ~21us of per-engine DMA work) starts ~79us and
   the kernel lands ~104us in the clean mode, ~110-118 in the slow
   environmental mode.
 - GpSimd does NO compute (only SWDGE store emission); scalar does
   all upsample copies; vector does pooling/topk/masks.
"""

import numpy as np

B, C, H, W = 8, 256, 132, 132
HW = H * W            # 17424
NB = 17               # 8x8 blocks per side (132 padded to 136)
NBLK = NB * NB        # 289
KEEP = 72             # round(289 * 0.25)
N_CORES = 8
S = (B * C) // N_CORES  # 256 slices per core
WP = W // 4           # 33 packed u32 per pixel row
LOAD_CHUNKS = ((0, 16), (16, 48), (48, 80), (80, 104), (104, 120),
               (120, 132))
# Store chunks per group: SWDGE descriptor emission is ~1us per DMA,
# serial on the Q7 core, so many small stores trickle descriptors and
# leave the SDMA engines idle at stream-start. g0's stores start while
# loads still run (granularity irrelevant) -> 2 big chunks; g1's first
# chunk is small so its store stream starts right after topk g1.
STORE_CHUNKS_BY_GROUP = (((0, 64), (64, 132)),
                         ((0, 16), (16, 64), (64, 132)))
NEG = -1.0e30
# f32 whose 4 bytes are each fp8e4m3(1.0) = 0x38; mask * PACK4 produces
# the packed 4-pixel fp8 row exactly (mask is exactly 0.0 or 1.0).
PACK4 = float(np.frombuffer(np.uint32(0x38383838).tobytes(),
                            dtype=np.float32)[0])

_prog_cache = {}


def _build_program():
    import concourse.bacc as bacc
    import concourse.mybir as mybir
    import concourse.tile as tile

    f32 = mybir.dt.float32
    fp8 = mybir.dt.float8e4
    X = mybir.AxisListType.X
    XY = mybir.AxisListType.XY
    ADD = mybir.AluOpType.add

    nc = bacc.Bacc("TRN2", debug=False, num_devices=N_CORES)
    x = nc.dram_tensor("x", (S, HW), f32, kind="ExternalInput")
    y = nc.dram_tensor("y", (S, HW), f32, kind="ExternalOutput")

    with tile.TileContext(nc) as tc:
        with (
            tc.tile_pool(name="big", bufs=2) as bigp,
            tc.tile_pool(name="med", bufs=2) as medp,
            tc.tile_pool(name="small", bufs=2) as smallp,
        ):
            # All load DMAs are emitted before any store DMA. Loads ride
            # the sync HWDGE queue (g0's first chunk on the scalar HWDGE
            # queue, measured fastest in v1); stores ride the Pool
            # (SWDGE) queue so load/store packets interleave round-robin
            # on each SDMA engine.
            xb = [bigp.tile([128, HW], f32, name=f"xb_g{g}", tag="xb")
                  for g in range(S // 128)]
            # Load order: g1's FIRST chunk goes second so it lands ~15us
            # and vector can pool it inside pool-g0's arrival gap (before
            # topk g0). That shaves one chunk off the post-topk-g0
            # pool-g1 stage, which is the serial chain that gates the g1
            # store stream (the tail of all DMA work).
            load_seq = ([(0, LOAD_CHUNKS[0]), (1, LOAD_CHUNKS[0])]
                        + [(0, ch) for ch in LOAD_CHUNKS[1:]]
                        + [(1, ch) for ch in LOAD_CHUNKS[1:]])
            for li, (g, (r0, r1)) in enumerate(load_seq):
                # Alternate the two HWDGE queues: per-DMA completion
                # overhead serializes within a queue but overlaps across
                # queues (measured 117us -> 97us for the same 24 DMAs).
                eng = nc.sync if li % 2 == 0 else nc.scalar
                p0 = g * 128
                eng.dma_start(out=xb[g][:, r0 * W:r1 * W],
                              in_=x[p0:p0 + 128, r0 * W:r1 * W])

            xv = [xt.rearrange("p (r w) -> p r w", w=W) for xt in xb]
            scores = [smallp.tile([128, NBLK], f32, name=f"scores_g{g}",
                                  tag="scores") for g in range(2)]
            sc3 = [s.rearrange("p (h t) -> p h t", t=NB) for s in scores]

            def pool_chunk(g, r0, r1):
                rr1 = min(r1, 128)
                nc.vector.tensor_reduce(
                    out=sc3[g][:, r0 // 8:rr1 // 8, 0:16],
                    in_=(xv[g][:, r0:rr1, 0:128]
                         .rearrange("p (h r) (q c) -> p h q r c",
                                    r=8, c=8)),
                    axis=XY, op=ADD, apply_absolute_value=True)

            def pool_edges(g):
                nc.vector.tensor_reduce(
                    out=sc3[g][:, 0:16, 16:17],
                    in_=(xv[g][:, 0:128, 128:132]
                         .rearrange("p (h r) c -> p h r c", r=8)),
                    axis=XY, op=ADD, apply_absolute_value=True)
                nc.vector.tensor_reduce(
                    out=sc3[g][:, 16:17, 0:16],
                    in_=(xv[g][:, 128:132, 0:128]
                         .rearrange("p r (q c) -> p q r c", c=8)),
                    axis=XY, op=ADD, apply_absolute_value=True)
                nc.vector.tensor_reduce(
                    out=sc3[g][:, 16:17, 16:17],
                    in_=xv[g][:, 128:132, 128:132].unsqueeze(1),
                    axis=XY, op=ADD, apply_absolute_value=True)

            def topk(g, first_m8):
                # Top-72 per partition: 9 rounds of max8 + match_replace.
                # match_replace replaces the first unmatched occurrence,
                # so ties resolve to the lowest index like jax.lax.top_k.
                for it in range(KEEP // 8):
                    m8 = first_m8 if it == 0 else smallp.tile(
                        [128, 8], f32, name=f"m8_g{g}i{it}", tag="m8")
                    nc.vector.max(out=m8[:, :], in_=scores[g][:, :])
                    nc.vector.match_replace(out=scores[g][:, :],
                                            in_to_replace=m8[:, :],
                                            in_values=scores[g][:, :],
                                            imm_value=NEG)

            def pm_rm(g):
                # Packed block mask: replaced entries are -1e30, so
                # (score < 0) * PACK4 writes 0x38383838 (4 fp8 ones) for
                # selected blocks, 0.0 for the rest. Then the packed
                # row-mask [p, h, 33]: one 132-px row (33 packed elems)
                # per block-row; blocks 0..15 span 2 packed elems each,
                # edge block 16 exactly 1 (pixels 128-131).
                pm = smallp.tile([128, NBLK], f32, name=f"pm_g{g}",
                                 tag="pm")
                nc.vector.tensor_scalar(out=pm[:, :], in0=scores[g][:, :],
                                        scalar1=0.0, scalar2=PACK4,
                                        op0=mybir.AluOpType.is_lt,
                                        op1=mybir.AluOpType.mult)
                pm3 = pm.rearrange("p (h t) -> p h t", t=NB)
                rm = medp.tile([128, NB * WP], f32, name=f"rm_g{g}",
                               tag="rm")
                rm3 = rm.rearrange("p (h w) -> p h w", w=WP)
                nc.vector.tensor_copy(
                    out=rm3[:, :, 0:32].rearrange("p h (q c) -> p h q c",
                                                  c=2),
                    in_=(pm3[:, :, 0:16].unsqueeze(3)
                         .broadcast_to((128, NB, 16, 2))))
                nc.vector.tensor_copy(
                    out=rm3[:, :, 32:33],
                    in_=pm3[:, :, 16:17])
                return pm, rm3

            def upsample_store(g, rm3):
                # Vertical 8x upsample into the packed mask tile (scalar),
                # then SWDGE cast-store (fp8 -> f32) per chunk.
                p0 = g * 128
                mk = medp.tile([128, NB * 8 * WP], f32, name=f"mk_g{g}",
                               tag="mk")
                mk4 = mk.rearrange("p (h r w) -> p h r w", r=8, w=WP)
                chunks = STORE_CHUNKS_BY_GROUP[g]
                for k, (r0, r1) in enumerate(chunks):
                    h0, h1 = r0 // 8, (r1 + 7) // 8
                    src = (rm3[:, h0:h1, :].unsqueeze(2)
                           .broadcast_to((128, h1 - h0, 8, WP)))
                    nc.scalar.copy(out=mk4[:, h0:h1, :, :], in_=src)
                    # Store rows r0:r1 (crop block-row 16 to 4 rows via
                    # the flat view of mk).
                    nc.gpsimd.dma_start(
                        out=y[p0:p0 + 128, r0 * W:r1 * W],
                        in_=mk[:, r0 * WP:r1 * WP].bitcast(fp8))

            # Vector-chain order: pool g0 c0 -> pool g1 c0 (fills the
            # arrival gap while g0's c1 is still loading) -> pool g0
            # rest -> topk g0 -> pm/rm g0 -> pool g1 rest -> topk g1 ->
            # pm/rm g1. Moving one g1 chunk ahead of topk g0 shortens
            # the post-topk-g0 serial stage that gates g1's stores.
            pool_chunk(0, *LOAD_CHUNKS[0])
            pool_chunk(1, *LOAD_CHUNKS[0])
            for (r0, r1) in LOAD_CHUNKS[1:]:
                pool_chunk(0, r0, r1)
            pool_edges(0)
            # Pin topk g0 after pool-g1-chunk0 even if the scheduler
            # reorders: the token reads chunk0's scores (RAW) and writes
            # into the first m8 tile (WAW with the first max8).
            m8_g0 = smallp.tile([128, 8], f32, name="m8_g0i0", tag="m8")
            nc.vector.tensor_copy(out=m8_g0[0:1, 0:1],
                                  in_=sc3[1][0:1, 0:1, 0:1])
            topk(0, m8_g0)
            pm0, rm3_0 = pm_rm(0)
            # Ordering token: pins g1's REMAINING pooling (block-rows
            # >= 2, flat 34:289) after g0's mask, otherwise the
            # scheduler interleaves the groups' pooling and delays g0's
            # mask by ~20us. Chunk0 (block-rows 0-1) stays exempt.
            nc.vector.tensor_copy(
                out=scores[1][0:1, 2 * NB:NBLK],
                in_=pm0[0:1, 0:1].broadcast_to((1, NBLK - 2 * NB)))
            for (r0, r1) in LOAD_CHUNKS[1:]:
                pool_chunk(1, r0, r1)
            pool_edges(1)
            m8_g1 = smallp.tile([128, 8], f32, name="m8_g1i0", tag="m8")
            topk(1, m8_g1)
            pm1, rm3_1 = pm_rm(1)

            upsample_store(0, rm3_0)
            upsample_store(1, rm3_1)
    nc.compile()
    return nc


def _ensure_ntff_hook_module():
    """bass_utils' trace path does `from antenv.axon_hooks import
    get_axon_ntff_profile_hook` — a module this image doesn't ship.
    Register an equivalent (ctypes into libaxon_pjrt.so, mirroring
    trn_boot._ntff_profile_via_ctypes) so BASS_TRACE=1 works; degrade
    to a None hook (trace skipped) when unavailable."""
    import sys
    import types

    try:
        import antenv.axon_hooks  # noqa: F401
        return
    except Exception:
        pass

    hook = None
    try:
        import contextlib
        import ctypes

        so_path = "/opt/axon/libaxon_pjrt.so"
        lib = ctypes.CDLL(so_path)
        if hasattr(lib, "axon_start_nrt_profile"):
            lib.axon_start_nrt_profile.argtypes = [
                ctypes.POINTER(ctypes.c_int64), ctypes.c_size_t]
            lib.axon_start_nrt_profile.restype = ctypes.c_int64
            lib.axon_stop_nrt_profile.argtypes = [ctypes.c_char_p]
            lib.axon_stop_nrt_profile.restype = ctypes.c_int64

            @contextlib.contextmanager
            def _hook(output_dir, device_ids):
                import jax
                jax.devices()
                if device_ids:
                    ids = (ctypes.c_int64 * len(device_ids))(*device_ids)
                    rc = lib.axon_start_nrt_profile(ids, len(device_ids))
                else:
                    rc = lib.axon_start_nrt_profile(None, 0)
                if rc != 0:
                    raise RuntimeError(f"axon_start_nrt_profile rc={rc}")
                try:
                    yield
                finally:
                    n = lib.axon_stop_nrt_profile(str(output_dir).encode())
                    print(f"ntff profile: {n} file(s) -> {output_dir}",
                          file=sys.stderr)

            hook = _hook
    except Exception:
        hook = None

    mod = types.ModuleType("antenv.axon_hooks")
    mod.get_axon_ntff_profile_hook = lambda: hook
    mod.set_axon_ntff_profile_hook = lambda h: None
    sys.modules["antenv.axon_hooks"] = mod


def _get_program():
    if "nc" not in _prog_cache:
        _prog_cache["nc"] = _build_program()
    return _prog_cache["nc"]


def kernel(features, enabled):
    feats = np.asarray(features)
    if not bool(np.asarray(enabled)):
        return np.ones(feats.shape, dtype=np.float32)

    _ensure_ntff_hook_module()
    import concourse.bass_utils as _bu
    from concourse.bass_utils import run_bass_kernel_spmd

    # The trace path uploads artifacts to a shared bucket; tolerate
    # sandboxes where that fails.
    if not getattr(_bu, "_upload_patched", False):
        _orig_upload = _bu.upload_artifacts

        def _safe_upload(tmpdir):
            try:
                return _orig_upload(tmpdir)
            except Exception:
                return str(tmpdir)

        _bu.upload_artifacts = _safe_upload
        _bu._upload_patched = True

    nc = _get_program()
    flat = np.ascontiguousarray(feats.reshape(B * C, HW), dtype=np.float32)
    in_maps = [{"x": flat[i * S:(i + 1) * S]} for i in range(N_CORES)]
    res = run_bass_kernel_spmd(nc, in_maps, list(range(N_CORES)))
    _prog_cache["last_res"] = res
    out = np.concatenate([np.asarray(res.results[i]["y"])
                          for i in range(N_CORES)], axis=0)
    return out.reshape(B, C, H, W).astype(np.float32)


# revision 21
# speedup vs baseline: 1.1384x; 1.1060x over previous
"""ChannelBlockImportanceGate kernel for 8 Trainium2 NeuronCores.

Computes, per (b, c) slice of features [8, 256, 132, 132] f32:
  scores = block-sum of |x| over 8x8 blocks (17x17 grid, zero-padded edges)
  top-72 blocks (ties -> lowest index, matching jax.lax.top_k)
  output = per-pixel {0,1} mask upsampled 8x8 (cropped to 132x132)

The straight-through soft term of the reference cancels in the forward
pass (hard - sg(soft) + soft == hard up to ~1ulp), so the output is the
hard mask.

Sharding: purely data parallel. 2048 (b,c) slices -> 256 per core.
Per core: 2 groups of 128 slices; each slice occupies one SBUF
partition so pooling/topk/upsample are per-partition ops with no
cross-partition traffic. Top-72 uses 9 rounds of DVE max8 +
match_replace(-1e30), then mask = (score < 0).

Performance model (from extensive perfetto analysis + microbenches):
 - The kernel is DMA-bound: 17.8MB load + 17.8MB store per core at
   the ~26.4GB/s per-SDMA-engine SBUF-port rate = ~84.5us of busy
   time on each of the 16 engines. A pure DMA round-trip of the same
   bytes measures 96-118us depending on an ENVIRONMENTAL mode (SDMA
   engine 15 sporadically runs ~20% slower, minutes-timescale; not
   controllable from the kernel). The full kernel measures within
   +-noise of that pure-DMA control, i.e. all compute is hidden.
 - Cast-during-DMA does NOT reduce engine busy time (engine time is
   set by the f32 side -- measured), but the PACKED fp8 mask makes
   the upsample 4x cheaper on the compute engines: one f32 element =
   4 fp8(0|1) pixels (mask * 0x38383838-as-f32, exact since mask is
   {0,1}); SWDGE stores cast fp8->f32 bit-exactly into the output.
 - DMAs are spread across all three queues (loads alternate the two
   HWDGE queues, stores ride the Pool/SWDGE queue): per-DMA
   completion overhead serializes within a queue but overlaps across
   queues (measured 117us -> 97us for the same 24 chunked DMAs).
 - Vector chain: pool g0 (18.7us, arrival-paced chunks) -> topk g0
   (9.5us) -> pool g1 -> topk g1, finishing ~77us so the g1 store
   stream (the last ~21us of per-engine DMA work) starts ~79us and
   the kernel lands ~104us in the clean mode, ~110-118 in the slow
   environmental mode.
 - GpSimd does NO compute (only SWDGE store emission); scalar does
   all upsample copies; vector does pooling/topk/masks.
"""

import numpy as np

B, C, H, W = 8, 256, 132, 132
HW = H * W            # 17424
NB = 17               # 8x8 blocks per side (132 padded to 136)
NBLK = NB * NB        # 289
KEEP = 72             # round(289 * 0.25)
N_CORES = 8
S = (B * C) // N_CORES  # 256 slices per core
WP = W // 4           # 33 packed u32 per pixel row
LOAD_CHUNKS = ((0, 16), (16, 48), (48, 80), (80, 104), (104, 120),
               (120, 132))
# Store chunks per group: SWDGE descriptor emission is ~1us per DMA,
# serial on the Q7 core, so many small stores trickle descriptors and
# leave the SDMA engines idle at stream-start. g0's stores start while
# loads still run (granularity irrelevant) -> 2 big chunks; g1's first
# chunk is small so its store stream starts right after topk g1.
STORE_CHUNKS_BY_GROUP = (((0, 64), (64, 132)),
                         ((0, 16), (16, 64), (64, 132)))
NEG = -1.0e30
# f32 whose 4 bytes are each fp8e4m3(1.0) = 0x38; mask * PACK4 produces
# the packed 4-pixel fp8 row exactly (mask is exactly 0.0 or 1.0).
PACK4 = float(np.frombuffer(np.uint32(0x38383838).tobytes(),
                            dtype=np.float32)[0])

_prog_cache = {}


def _build_program():
    import concourse.bacc as bacc
    import concourse.mybir as mybir
    import concourse.tile as tile

    f32 = mybir.dt.float32
    fp8 = mybir.dt.float8e4
    X = mybir.AxisListType.X
    XY = mybir.AxisListType.XY
    ADD = mybir.AluOpType.add

    nc = bacc.Bacc("TRN2", debug=False, num_devices=N_CORES)
    x = nc.dram_tensor("x", (S, HW), f32, kind="ExternalInput")
    y = nc.dram_tensor("y", (S, HW), f32, kind="ExternalOutput")

    with tile.TileContext(nc) as tc:
        with (
            tc.tile_pool(name="big", bufs=2) as bigp,
            tc.tile_pool(name="med", bufs=2) as medp,
            tc.tile_pool(name="small", bufs=2) as smallp,
        ):
            # All load DMAs are emitted before any store DMA. Loads ride
            # the sync HWDGE queue (g0's first chunk on the scalar HWDGE
            # queue, measured fastest in v1); stores ride the Pool
            # (SWDGE) queue so load/store packets interleave round-robin
            # on each SDMA engine.
            xb = [bigp.tile([128, HW], f32, name=f"xb_g{g}", tag="xb")
                  for g in range(S // 128)]
            load_seq = [(g, ch) for g in range(2) for ch in LOAD_CHUNKS]
            for li, (g, (r0, r1)) in enumerate(load_seq):
                # Alternate the two HWDGE queues: per-DMA completion
                # overhead serializes within a queue but overlaps across
                # queues (measured 117us -> 97us for the same 24 DMAs).
                eng = nc.sync if li % 2 == 0 else nc.scalar
                p0 = g * 128
                eng.dma_start(out=xb[g][:, r0 * W:r1 * W],
                              in_=x[p0:p0 + 128, r0 * W:r1 * W])

            xv = [xt.rearrange("p (r w) -> p r w", w=W) for xt in xb]
            scores = [smallp.tile([128, NBLK], f32, name=f"scores_g{g}",
                                  tag="scores") for g in range(2)]
            sc3 = [s.rearrange("p (h t) -> p h t", t=NB) for s in scores]

            def pool_chunk(g, r0, r1):
                rr1 = min(r1, 128)
                nc.vector.tensor_reduce(
                    out=sc3[g][:, r0 // 8:rr1 // 8, 0:16],
                    in_=(xv[g][:, r0:rr1, 0:128]
                         .rearrange("p (h r) (q c) -> p h q r c",
                                    r=8, c=8)),
                    axis=XY, op=ADD, apply_absolute_value=True)

            def pool_edges(g):
                nc.vector.tensor_reduce(
                    out=sc3[g][:, 0:16, 16:17],
                    in_=(xv[g][:, 0:128, 128:132]
                         .rearrange("p (h r) c -> p h r c", r=8)),
                    axis=XY, op=ADD, apply_absolute_value=True)
                nc.vector.tensor_reduce(
                    out=sc3[g][:, 16:17, 0:16],
                    in_=(xv[g][:, 128:132, 0:128]
                         .rearrange("p r (q c) -> p q r c", c=8)),
                    axis=XY, op=ADD, apply_absolute_value=True)
                nc.vector.tensor_reduce(
                    out=sc3[g][:, 16:17, 16:17],
                    in_=xv[g][:, 128:132, 128:132].unsqueeze(1),
                    axis=XY, op=ADD, apply_absolute_value=True)

            def topk(g, first_m8):
                # Top-72 per partition: 9 rounds of max8 + match_replace.
                # match_replace replaces the first unmatched occurrence,
                # so ties resolve to the lowest index like jax.lax.top_k.
                for it in range(KEEP // 8):
                    m8 = first_m8 if it == 0 else smallp.tile(
                        [128, 8], f32, name=f"m8_g{g}i{it}", tag="m8")
                    nc.vector.max(out=m8[:, :], in_=scores[g][:, :])
                    nc.vector.match_replace(out=scores[g][:, :],
                                            in_to_replace=m8[:, :],
                                            in_values=scores[g][:, :],
                                            imm_value=NEG)

            def pm_rm(g):
                # Packed block mask: replaced entries are -1e30, so
                # (score < 0) * PACK4 writes 0x38383838 (4 fp8 ones) for
                # selected blocks, 0.0 for the rest. Then the packed
                # row-mask [p, h, 33]: one 132-px row (33 packed elems)
                # per block-row; blocks 0..15 span 2 packed elems each,
                # edge block 16 exactly 1 (pixels 128-131).
                pm = smallp.tile([128, NBLK], f32, name=f"pm_g{g}",
                                 tag="pm")
                nc.vector.tensor_scalar(out=pm[:, :], in0=scores[g][:, :],
                                        scalar1=0.0, scalar2=PACK4,
                                        op0=mybir.AluOpType.is_lt,
                                        op1=mybir.AluOpType.mult)
                pm3 = pm.rearrange("p (h t) -> p h t", t=NB)
                rm = medp.tile([128, NB * WP], f32, name=f"rm_g{g}",
                               tag="rm")
                rm3 = rm.rearrange("p (h w) -> p h w", w=WP)
                nc.vector.tensor_copy(
                    out=rm3[:, :, 0:32].rearrange("p h (q c) -> p h q c",
                                                  c=2),
                    in_=(pm3[:, :, 0:16].unsqueeze(3)
                         .broadcast_to((128, NB, 16, 2))))
                nc.vector.tensor_copy(
                    out=rm3[:, :, 32:33],
                    in_=pm3[:, :, 16:17])
                return pm, rm3

            def upsample_store(g, rm3):
                # Vertical 8x upsample into the packed mask tile (scalar),
                # then SWDGE cast-store (fp8 -> f32) per chunk.
                p0 = g * 128
                mk = medp.tile([128, NB * 8 * WP], f32, name=f"mk_g{g}",
                               tag="mk")
                mk4 = mk.rearrange("p (h r w) -> p h r w", r=8, w=WP)
                chunks = STORE_CHUNKS_BY_GROUP[g]
                for k, (r0, r1) in enumerate(chunks):
                    h0, h1 = r0 // 8, (r1 + 7) // 8
                    src = (rm3[:, h0:h1, :].unsqueeze(2)
                           .broadcast_to((128, h1 - h0, 8, WP)))
                    nc.scalar.copy(out=mk4[:, h0:h1, :, :], in_=src)
                    # Store rows r0:r1 (crop block-row 16 to 4 rows via
                    # the flat view of mk).
                    nc.gpsimd.dma_start(
                        out=y[p0:p0 + 128, r0 * W:r1 * W],
                        in_=mk[:, r0 * WP:r1 * WP].bitcast(fp8))

            # Vector-chain order: pool g0 (arrival-paced) -> topk g0 ->
            # pm/rm g0 -> pool g1 -> topk g1 -> pm/rm g1. (Hoisting g1
            # chunks ahead of topk g0 was tried and measured neutral to
            # worse: the scheduler controls same-engine placement and
            # the load reorder delays g0's mask in slow windows.)
            for (r0, r1) in LOAD_CHUNKS:
                pool_chunk(0, r0, r1)
            pool_edges(0)
            m8_g0 = smallp.tile([128, 8], f32, name="m8_g0i0", tag="m8")
            topk(0, m8_g0)
            pm0, rm3_0 = pm_rm(0)
            # Ordering token: pins g1's pooling after g0's mask on the
            # vector engine, otherwise the scheduler interleaves the two
            # groups' pooling and delays g0's mask by ~20us.
            nc.vector.tensor_copy(
                out=scores[1][0:1, :],
                in_=pm0[0:1, 0:1].broadcast_to((1, NBLK)))
            for (r0, r1) in LOAD_CHUNKS:
                pool_chunk(1, r0, r1)
            pool_edges(1)
            m8_g1 = smallp.tile([128, 8], f32, name="m8_g1i0", tag="m8")
            topk(1, m8_g1)
            pm1, rm3_1 = pm_rm(1)

            upsample_store(0, rm3_0)
            upsample_store(1, rm3_1)
    nc.compile()
    return nc


def _ensure_ntff_hook_module():
    """bass_utils' trace path does `from antenv.axon_hooks import
    get_axon_ntff_profile_hook` — a module this image doesn't ship.
    Register an equivalent (ctypes into libaxon_pjrt.so, mirroring
    trn_boot._ntff_profile_via_ctypes) so BASS_TRACE=1 works; degrade
    to a None hook (trace skipped) when unavailable."""
    import sys
    import types

    try:
        import antenv.axon_hooks  # noqa: F401
        return
    except Exception:
        pass

    hook = None
    try:
        import contextlib
        import ctypes

        so_path = "/opt/axon/libaxon_pjrt.so"
        lib = ctypes.CDLL(so_path)
        if hasattr(lib, "axon_start_nrt_profile"):
            lib.axon_start_nrt_profile.argtypes = [
                ctypes.POINTER(ctypes.c_int64), ctypes.c_size_t]
            lib.axon_start_nrt_profile.restype = ctypes.c_int64
            lib.axon_stop_nrt_profile.argtypes = [ctypes.c_char_p]
            lib.axon_stop_nrt_profile.restype = ctypes.c_int64

            @contextlib.contextmanager
            def _hook(output_dir, device_ids):
                import jax
                jax.devices()
                if device_ids:
                    ids = (ctypes.c_int64 * len(device_ids))(*device_ids)
                    rc = lib.axon_start_nrt_profile(ids, len(device_ids))
                else:
                    rc = lib.axon_start_nrt_profile(None, 0)
                if rc != 0:
                    raise RuntimeError(f"axon_start_nrt_profile rc={rc}")
                try:
                    yield
                finally:
                    n = lib.axon_stop_nrt_profile(str(output_dir).encode())
                    print(f"ntff profile: {n} file(s) -> {output_dir}",
                          file=sys.stderr)

            hook = _hook
    except Exception:
        hook = None

    mod = types.ModuleType("antenv.axon_hooks")
    mod.get_axon_ntff_profile_hook = lambda: hook
    mod.set_axon_ntff_profile_hook = lambda h: None
    sys.modules["antenv.axon_hooks"] = mod


def _get_program():
    if "nc" not in _prog_cache:
        _prog_cache["nc"] = _build_program()
    return _prog_cache["nc"]


def kernel(features, enabled):
    feats = np.asarray(features)
    if not bool(np.asarray(enabled)):
        return np.ones(feats.shape, dtype=np.float32)

    _ensure_ntff_hook_module()
    import concourse.bass_utils as _bu
    from concourse.bass_utils import run_bass_kernel_spmd

    # The trace path uploads artifacts to a shared bucket; tolerate
    # sandboxes where that fails.
    if not getattr(_bu, "_upload_patched", False):
        _orig_upload = _bu.upload_artifacts

        def _safe_upload(tmpdir):
            try:
                return _orig_upload(tmpdir)
            except Exception:
                return str(tmpdir)

        _bu.upload_artifacts = _safe_upload
        _bu._upload_patched = True

    nc = _get_program()
    flat = np.ascontiguousarray(feats.reshape(B * C, HW), dtype=np.float32)
    in_maps = [{"x": flat[i * S:(i + 1) * S]} for i in range(N_CORES)]
    res = run_bass_kernel_spmd(nc, in_maps, list(range(N_CORES)))
    _prog_cache["last_res"] = res
    out = np.concatenate([np.asarray(res.results[i]["y"])
                          for i in range(N_CORES)], axis=0)
    return out.reshape(B, C, H, W).astype(np.float32)


# revision 22
# speedup vs baseline: 1.1413x; 1.0026x over previous
"""ChannelBlockImportanceGate kernel for 8 Trainium2 NeuronCores.

Computes, per (b, c) slice of features [8, 256, 132, 132] f32:
  scores = block-sum of |x| over 8x8 blocks (17x17 grid, zero-padded edges)
  top-72 blocks (ties -> lowest index, matching jax.lax.top_k)
  output = per-pixel {0,1} mask upsampled 8x8 (cropped to 132x132)

The straight-through soft term of the reference cancels in the forward
pass (hard - sg(soft) + soft == hard up to ~1ulp), so the output is the
hard mask.

Sharding: purely data parallel. 2048 (b,c) slices -> 256 per core.
Per core: 2 groups of 128 slices; each slice occupies one SBUF
partition so pooling/topk/upsample are per-partition ops with no
cross-partition traffic. Top-72 uses 9 rounds of DVE max8 +
match_replace(-1e30), then mask = (score < 0).

Performance model (from extensive perfetto analysis + microbenches):
 - The kernel is DMA-bound: 17.8MB load + 17.8MB store per core at
   the ~26.4GB/s per-SDMA-engine SBUF-port rate = ~84.5us of busy
   time on each of the 16 engines. A pure DMA round-trip of the same
   bytes measures 96-118us depending on an ENVIRONMENTAL mode (SDMA
   engine 15 sporadically runs ~20% slower, minutes-timescale; not
   controllable from the kernel). The full kernel measures within
   +-noise of that pure-DMA control, i.e. all compute is hidden.
 - Cast-during-DMA does NOT reduce engine busy time (engine time is
   set by the f32 side -- measured), but the PACKED fp8 mask makes
   the upsample 4x cheaper on the compute engines: one f32 element =
   4 fp8(0|1) pixels (mask * 0x38383838-as-f32, exact since mask is
   {0,1}); SWDGE stores cast fp8->f32 bit-exactly into the output.
 - DMAs are spread across all three queues (loads alternate the two
   HWDGE queues, stores ride the Pool/SWDGE queue): per-DMA
   completion overhead serializes within a queue but overlaps across
   queues (measured 117us -> 97us for the same 24 chunked DMAs).
 - Vector chain: pool g0 (18.7us, arrival-paced chunks) -> topk g0
   (9.5us) -> pool g1 -> topk g1, finishing ~77us so the g1 store
   stream (the last ~21us of per-engine DMA work) starts ~79us and
   the kernel lands ~104us in the clean mode, ~110-118 in the slow
   environmental mode.
 - GpSimd does NO compute (only SWDGE store emission); scalar does
   all upsample copies; vector does pooling/topk/masks.
"""

import numpy as np

B, C, H, W = 8, 256, 132, 132
HW = H * W            # 17424
NB = 17               # 8x8 blocks per side (132 padded to 136)
NBLK = NB * NB        # 289
KEEP = 72             # round(289 * 0.25)
N_CORES = 8
S = (B * C) // N_CORES  # 256 slices per core
WP = W // 4           # 33 packed u32 per pixel row
LOAD_CHUNKS = ((0, 16), (16, 48), (48, 80), (80, 104), (104, 120),
               (120, 132))
# Store chunks per group: SWDGE descriptor emission is ~1us per DMA,
# serial on the Q7 core, so many small stores trickle descriptors and
# leave the SDMA engines idle at stream-start. g0's stores start while
# loads still run (granularity irrelevant) -> 2 big chunks; g1's first
# chunk is small so its store stream starts right after topk g1.
STORE_CHUNKS_BY_GROUP = (((0, 64), (64, 132)),
                         ((0, 16), (16, 64), (64, 132)))
# BRIDGE variant: split g0's stores 3-way and hold the last ~5us of its
# store work behind g1's mask to keep the DMA engines fed through the
# g0->g1 store handoff; also upsample g1's first chunk on vector right
# after rm (saves a cross-engine sem hop before the first g1 store).
BRIDGE = False
STORE_CHUNKS_G0_BRIDGE = ((0, 64), (64, 100), (100, 132))
NEG = -1.0e30
# f32 whose 4 bytes are each fp8e4m3(1.0) = 0x38; mask * PACK4 produces
# the packed 4-pixel fp8 row exactly (mask is exactly 0.0 or 1.0).
PACK4 = float(np.frombuffer(np.uint32(0x38383838).tobytes(),
                            dtype=np.float32)[0])

_prog_cache = {}


def _build_program():
    import concourse.bacc as bacc
    import concourse.mybir as mybir
    import concourse.tile as tile

    f32 = mybir.dt.float32
    fp8 = mybir.dt.float8e4
    X = mybir.AxisListType.X
    XY = mybir.AxisListType.XY
    ADD = mybir.AluOpType.add

    nc = bacc.Bacc("TRN2", debug=False, num_devices=N_CORES)
    x = nc.dram_tensor("x", (S, HW), f32, kind="ExternalInput")
    y = nc.dram_tensor("y", (S, HW), f32, kind="ExternalOutput")

    with tile.TileContext(nc) as tc:
        with (
            tc.tile_pool(name="big", bufs=2) as bigp,
            tc.tile_pool(name="med", bufs=2) as medp,
            tc.tile_pool(name="small", bufs=2) as smallp,
        ):
            # All load DMAs are emitted before any store DMA. Loads ride
            # the sync HWDGE queue (g0's first chunk on the scalar HWDGE
            # queue, measured fastest in v1); stores ride the Pool
            # (SWDGE) queue so load/store packets interleave round-robin
            # on each SDMA engine.
            xb = [bigp.tile([128, HW], f32, name=f"xb_g{g}", tag="xb")
                  for g in range(S // 128)]
            load_seq = [(g, ch) for g in range(2) for ch in LOAD_CHUNKS]
            for li, (g, (r0, r1)) in enumerate(load_seq):
                # Alternate the two HWDGE queues: per-DMA completion
                # overhead serializes within a queue but overlaps across
                # queues (measured 117us -> 97us for the same 24 DMAs).
                eng = nc.sync if li % 2 == 0 else nc.scalar
                p0 = g * 128
                eng.dma_start(out=xb[g][:, r0 * W:r1 * W],
                              in_=x[p0:p0 + 128, r0 * W:r1 * W])

            xv = [xt.rearrange("p (r w) -> p r w", w=W) for xt in xb]
            scores = [smallp.tile([128, NBLK], f32, name=f"scores_g{g}",
                                  tag="scores") for g in range(2)]
            sc3 = [s.rearrange("p (h t) -> p h t", t=NB) for s in scores]

            def pool_chunk(g, r0, r1):
                rr1 = min(r1, 128)
                nc.vector.tensor_reduce(
                    out=sc3[g][:, r0 // 8:rr1 // 8, 0:16],
                    in_=(xv[g][:, r0:rr1, 0:128]
                         .rearrange("p (h r) (q c) -> p h q r c",
                                    r=8, c=8)),
                    axis=XY, op=ADD, apply_absolute_value=True)

            def pool_edges(g):
                nc.vector.tensor_reduce(
                    out=sc3[g][:, 0:16, 16:17],
                    in_=(xv[g][:, 0:128, 128:132]
                         .rearrange("p (h r) c -> p h r c", r=8)),
                    axis=XY, op=ADD, apply_absolute_value=True)
                nc.vector.tensor_reduce(
                    out=sc3[g][:, 16:17, 0:16],
                    in_=(xv[g][:, 128:132, 0:128]
                         .rearrange("p r (q c) -> p q r c", c=8)),
                    axis=XY, op=ADD, apply_absolute_value=True)
                nc.vector.tensor_reduce(
                    out=sc3[g][:, 16:17, 16:17],
                    in_=xv[g][:, 128:132, 128:132].unsqueeze(1),
                    axis=XY, op=ADD, apply_absolute_value=True)

            def topk(g, first_m8):
                # Top-72 per partition: 9 rounds of max8 + match_replace.
                # match_replace replaces the first unmatched occurrence,
                # so ties resolve to the lowest index like jax.lax.top_k.
                for it in range(KEEP // 8):
                    m8 = first_m8 if it == 0 else smallp.tile(
                        [128, 8], f32, name=f"m8_g{g}i{it}", tag="m8")
                    nc.vector.max(out=m8[:, :], in_=scores[g][:, :])
                    nc.vector.match_replace(out=scores[g][:, :],
                                            in_to_replace=m8[:, :],
                                            in_values=scores[g][:, :],
                                            imm_value=NEG)

            def pm_rm(g):
                # Packed block mask: replaced entries are -1e30, so
                # (score < 0) * PACK4 writes 0x38383838 (4 fp8 ones) for
                # selected blocks, 0.0 for the rest. Then the packed
                # row-mask [p, h, 33]: one 132-px row (33 packed elems)
                # per block-row; blocks 0..15 span 2 packed elems each,
                # edge block 16 exactly 1 (pixels 128-131).
                pm = smallp.tile([128, NBLK], f32, name=f"pm_g{g}",
                                 tag="pm")
                nc.vector.tensor_scalar(out=pm[:, :], in0=scores[g][:, :],
                                        scalar1=0.0, scalar2=PACK4,
                                        op0=mybir.AluOpType.is_lt,
                                        op1=mybir.AluOpType.mult)
                pm3 = pm.rearrange("p (h t) -> p h t", t=NB)
                rm = medp.tile([128, NB * WP], f32, name=f"rm_g{g}",
                               tag="rm")
                rm3 = rm.rearrange("p (h w) -> p h w", w=WP)
                nc.vector.tensor_copy(
                    out=rm3[:, :, 0:32].rearrange("p h (q c) -> p h q c",
                                                  c=2),
                    in_=(pm3[:, :, 0:16].unsqueeze(3)
                         .broadcast_to((128, NB, 16, 2))))
                nc.vector.tensor_copy(
                    out=rm3[:, :, 32:33],
                    in_=pm3[:, :, 16:17])
                return pm, rm3

            def upsample_store(g, rm3, hold_on=None):
                # Vertical 8x upsample into the packed mask tile (scalar),
                # then SWDGE cast-store (fp8 -> f32) per chunk.
                p0 = g * 128
                mk = medp.tile([128, NB * 8 * WP], f32, name=f"mk_g{g}",
                               tag="mk")
                mk4 = mk.rearrange("p (h r w) -> p h r w", r=8, w=WP)
                if g == 0 and BRIDGE:
                    chunks = STORE_CHUNKS_G0_BRIDGE
                else:
                    chunks = STORE_CHUNKS_BY_GROUP[g]
                for k, (r0, r1) in enumerate(chunks):
                    h0, h1 = r0 // 8, (r1 + 7) // 8
                    src = (rm3[:, h0:h1, :].unsqueeze(2)
                           .broadcast_to((128, h1 - h0, 8, WP)))
                    if hold_on is not None and k == len(chunks) - 1:
                        # 1-elem token: RAW on g1's mask, WAW with this
                        # chunk's upsample copy -> the held store only
                        # executes once g1's stores are nearly ready.
                        nc.scalar.copy(
                            out=mk[0:1, r0 * WP:r0 * WP + 1],
                            in_=hold_on[0:1, 0:1])
                    if BRIDGE and g == 1 and k == 0:
                        # First g1 chunk on vector, right after rm: the
                        # store stream starts without waiting a scalar
                        # sem hop.
                        nc.vector.tensor_copy(out=mk4[:, h0:h1, :, :],
                                              in_=src)
                    else:
                        nc.scalar.copy(out=mk4[:, h0:h1, :, :], in_=src)
                    # Store rows r0:r1 (crop block-row 16 to 4 rows via
                    # the flat view of mk).
                    nc.gpsimd.dma_start(
                        out=y[p0:p0 + 128, r0 * W:r1 * W],
                        in_=mk[:, r0 * WP:r1 * WP].bitcast(fp8))

            # Vector-chain order: pool g0 (arrival-paced) -> topk g0 ->
            # pm/rm g0 -> pool g1 -> topk g1 -> pm/rm g1. (Hoisting g1
            # chunks ahead of topk g0 was tried and measured neutral to
            # worse: the scheduler controls same-engine placement and
            # the load reorder delays g0's mask in slow windows.)
            for (r0, r1) in LOAD_CHUNKS:
                pool_chunk(0, r0, r1)
            pool_edges(0)
            m8_g0 = smallp.tile([128, 8], f32, name="m8_g0i0", tag="m8")
            topk(0, m8_g0)
            pm0, rm3_0 = pm_rm(0)
            # Ordering token: pins g1's pooling after g0's mask on the
            # vector engine, otherwise the scheduler interleaves the two
            # groups' pooling and delays g0's mask by ~20us.
            nc.vector.tensor_copy(
                out=scores[1][0:1, :],
                in_=pm0[0:1, 0:1].broadcast_to((1, NBLK)))
            for (r0, r1) in LOAD_CHUNKS:
                pool_chunk(1, r0, r1)
            pool_edges(1)
            m8_g1 = smallp.tile([128, 8], f32, name="m8_g1i0", tag="m8")
            topk(1, m8_g1)
            pm1, rm3_1 = pm_rm(1)

            upsample_store(0, rm3_0,
                           hold_on=pm1 if BRIDGE else None)
            upsample_store(1, rm3_1)
    nc.compile()
    return nc


def _ensure_ntff_hook_module():
    """bass_utils' trace path does `from antenv.axon_hooks import
    get_axon_ntff_profile_hook` — a module this image doesn't ship.
    Register an equivalent (ctypes into libaxon_pjrt.so, mirroring
    trn_boot._ntff_profile_via_ctypes) so BASS_TRACE=1 works; degrade
    to a None hook (trace skipped) when unavailable."""
    import sys
    import types

    try:
        import antenv.axon_hooks  # noqa: F401
        return
    except Exception:
        pass

    hook = None
    try:
        import contextlib
        import ctypes

        so_path = "/opt/axon/libaxon_pjrt.so"
        lib = ctypes.CDLL(so_path)
        if hasattr(lib, "axon_start_nrt_profile"):
            lib.axon_start_nrt_profile.argtypes = [
                ctypes.POINTER(ctypes.c_int64), ctypes.c_size_t]
            lib.axon_start_nrt_profile.restype = ctypes.c_int64
            lib.axon_stop_nrt_profile.argtypes = [ctypes.c_char_p]
            lib.axon_stop_nrt_profile.restype = ctypes.c_int64

            @contextlib.contextmanager
            def _hook(output_dir, device_ids):
                import jax
                jax.devices()
                if device_ids:
                    ids = (ctypes.c_int64 * len(device_ids))(*device_ids)
                    rc = lib.axon_start_nrt_profile(ids, len(device_ids))
                else:
                    rc = lib.axon_start_nrt_profile(None, 0)
                if rc != 0:
                    raise RuntimeError(f"axon_start_nrt_profile rc={rc}")
                try:
                    yield
                finally:
                    n = lib.axon_stop_nrt_profile(str(output_dir).encode())
                    print(f"ntff profile: {n} file(s) -> {output_dir}",
                          file=sys.stderr)

            hook = _hook
    except Exception:
        hook = None

    mod = types.ModuleType("antenv.axon_hooks")
    mod.get_axon_ntff_profile_hook = lambda: hook
    mod.set_axon_ntff_profile_hook = lambda h: None
    sys.modules["antenv.axon_hooks"] = mod


def _get_program():
    if "nc" not in _prog_cache:
        _prog_cache["nc"] = _build_program()
    return _prog_cache["nc"]


def kernel(features, enabled):
    feats = np.asarray(features)
    if not bool(np.asarray(enabled)):
        return np.ones(feats.shape, dtype=np.float32)

    _ensure_ntff_hook_module()
    import concourse.bass_utils as _bu
    from concourse.bass_utils import run_bass_kernel_spmd

    # The trace path uploads artifacts to a shared bucket; tolerate
    # sandboxes where that fails.
    if not getattr(_bu, "_upload_patched", False):
        _orig_upload = _bu.upload_artifacts

        def _safe_upload(tmpdir):
            try:
                return _orig_upload(tmpdir)
            except Exception:
                return str(tmpdir)

        _bu.upload_artifacts = _safe_upload
        _bu._upload_patched = True

    nc = _get_program()
    flat = np.ascontiguousarray(feats.reshape(B * C, HW), dtype=np.float32)
    in_maps = [{"x": flat[i * S:(i + 1) * S]} for i in range(N_CORES)]
    res = run_bass_kernel_spmd(nc, in_maps, list(range(N_CORES)))
    _prog_cache["last_res"] = res
    out = np.concatenate([np.asarray(res.results[i]["y"])
                          for i in range(N_CORES)], axis=0)
    return out.reshape(B, C, H, W).astype(np.float32)
